# revision 56
# baseline (speedup 1.0000x reference)
"""CBTree (levelized complete 4-ary tree, depth 8, D=512) forward on 8 TRN2 NeuronCores.

Strategy
--------
Math: per level,  h = tanh(x + sum_b lc[b]*(h_b @ wl.T) + rc[b]*(h_b @ wr.T)).
By linearity the per-child matmuls collapse to two matmuls of weighted child
sums:  agg = u_l' @ (wl.T/3) + u_r' @ (wr.T/3)  with integer-coefficient sums
u_l' = 3 h0 + 2 h1 + h2 and u_r' = h1 + 2 h2 + 3 h3  (4x FLOP reduction).

Sharding: the 16 level-2 subtrees are sharded 2-per-core (b-major order so the
post-gather tail stays contiguous); each core runs levels 7..2 of its forest
locally, one AllGather collects the 16 level-2 hiddens, and every core
redundantly computes the tiny levels 1+0 tail.

Layout: everything on-chip is feature-major ([128 partitions, 4 d-tiles, n])
and node storage at every level is digit-reversed (base-4), which makes each
child block b a contiguous column range. The host pre-permutes/transposes/
casts inputs accordingly. x is injected into PSUM via an identity-weight
matmul so ScalarE only has to run tanh(PSUM)->SBUF.

Precision modes (PE runs fp16/bf16 at 1 cyc/row, fp32 at 4):
  f16   - everything fp16 (fastest, rel err ~1e-3)
  bf16  - everything bf16 (same speed, rel err ~9e-3)
  f16x  - fp32 storage, matmuls as 3-term split-fp16 products
          (U ~ Uhi+Ulo, W ~ Whi+Wlo; drop Ulo*Wlo): rel err ~3e-6
  fp32  - native fp32 matmuls: rel err ~5e-7
"""
import sys

import numpy as np
import ml_dtypes

sys.path.insert(0, "/opt/trn_rl_repo")

import concourse.bass as bass  # noqa: E402,F401
import concourse.bacc as bacc  # noqa: E402
import concourse.tile as tile  # noqa: E402
from concourse import mybir  # noqa: E402
from concourse.bass_utils import run_bass_kernel_spmd  # noqa: E402

NCORES = 8
D = 512
NT = 4  # d-tiles of 128
B = 4
DEPTH = 8
# local levels L=0..6 <-> global levels 2..8 ; per-core node counts
NLOC = [2 * 4**l for l in range(7)]  # [2, 8, 32, 128, 512, 2048, 8192]
CHUNK = 512
L5CW = 256

# precision of the on-device pipeline: "f16" | "bf16" | "f16x" | "fp32"
PRECISION = "f16"

BF16np = ml_dtypes.bfloat16
FP32 = mybir.dt.float32
BF16 = mybir.dt.bfloat16
F16 = mybir.dt.float16


# ---------------------------------------------------------------- host helpers
def _rev4(idx: np.ndarray, ndig: int) -> np.ndarray:
    r = np.zeros_like(idx)
    q = idx.copy()
    for _ in range(ndig):
        r = (r << 2) | (q & 3)
        q >>= 2
    return r


def _storage_nodes(level: int) -> np.ndarray:
    """Local node index stored at each storage column of local level ``level``.

    Local node j = r*4^L + q (r = which of the core's two subtrees) is stored
    at column 2*rev4(q) + r; returns the inverse map.
    """
    n = NLOC[level]
    j = np.arange(n, dtype=np.int64)
    r, q = j >> (2 * level), j & ((1 << (2 * level)) - 1)
    pos = 2 * _rev4(q, level) + r
    inv = np.empty(n, dtype=np.int64)
    inv[pos] = j
    return inv


def _mode_cfg():
    """-> (split, DT storage, MT matmul-operand, np storage, np mm, np x)."""
    m = PRECISION
    if m == "f16":
        return False, F16, F16, np.float16, np.float16, np.float16
    if m == "bf16":
        return False, BF16, BF16, BF16np, BF16np, BF16np
    if m == "f16x":
        return True, FP32, F16, np.float32, np.float16, np.float32
    return False, FP32, FP32, np.float32, np.float32, np.float32


def _hi_lo(a: np.ndarray, dt):
    hi = a.astype(dt)
    lo = (a - hi.astype(np.float32)).astype(dt)
    return hi, lo


# ---------------------------------------------------------------- device build
def _build_nc(with_tail=True):
    split, DT, MT, _, _, _ = _mode_cfg()
    nc = bacc.Bacc(
        "TRN2", target_bir_lowering=False, debug=False, num_devices=NCORES
    )

    leaf_d = [
        nc.dram_tensor(f"leaf{b}", [D, NLOC[6] // 4], DT, kind="ExternalInput")
        for b in range(B)
    ]

    # in split mode x stays a single fp32 tensor: the DVE adds it into PSUM
    # (no PE identity-injection, no hi/lo marshalling)
    XDT = FP32 if split else DT

    def xin(name, n):
        return [nc.dram_tensor(name, [D, n], XDT, kind="ExternalInput")]

    x_d = {L: xin(f"x{L}", NLOC[L]) for L in range(6)}
    xt1_d = xin("xt1", 4)
    xt0nm_d = (nc.dram_tensor("xt0nm", [1, D], FP32, kind="ExternalInput")
               if split else None)
    xt0_d = None if split else xin("xt0", 1)
    if split:
        w2_d = [
            nc.dram_tensor("w2hi", [2 * D, D], MT, kind="ExternalInput"),
            nc.dram_tensor("w2lo", [2 * D, D], MT, kind="ExternalInput"),
        ]
    else:
        w2_d = [nc.dram_tensor("w2", [2 * D, D], MT, kind="ExternalInput")]
    idm_d = nc.dram_tensor("identm", [128, 128], MT, kind="ExternalInput")
    out_d = nc.dram_tensor("out", [1, D], FP32, kind="ExternalOutput")

    def fm_ap(handle):
        # DRAM [512, n] -> [128p, 4t, n] with feature d = t*128 + p
        return handle.ap().rearrange("(t p) n -> p t n", p=128)

    mult, add = mybir.AluOpType.mult, mybir.AluOpType.add
    Tanh = mybir.ActivationFunctionType.Tanh

    with tile.TileContext(nc) as tc:
        with (
            tc.tile_pool(name="const", bufs=1) as const,
            tc.tile_pool(name="hp", bufs=1) as hp,
            tc.tile_pool(name="leafp", bufs=8) as leafp,
            tc.tile_pool(name="up", bufs=6 if DT != FP32 else 2) as up,
            tc.tile_pool(name="usp", bufs=2) as usp,
            tc.tile_pool(name="tmpp", bufs=6 if DT != FP32 else 2) as tmpp,
            tc.tile_pool(name="xsp", bufs=3) as xsp,
            tc.tile_pool(name="psum", bufs=8, space="PSUM") as psum,
            tc.tile_pool(name="dram", bufs=1, space="DRAM") as dram,
        ):
            # prefetch the first leaf chunk + its x slice before anything else
            # so the DVE/PE pipeline starts as early as possible
            pre_lts = []
            for b in range(B):
                lt = leafp.tile([128, NT, L5CW], DT, tag="lb", name=f"plb{b}")
                nc.sync.dma_start(out=lt[:], in_=fm_ap(leaf_d[b])[:, :, 0:L5CW])
                pre_lts.append(lt)
            pre_xt = []
            for i, xd in enumerate(x_d[5]):
                t = xsp.tile([128, NT, L5CW], XDT,
                             tag=f"xs{i}", name=f"pxs{i}")
                nc.sync.dma_start(out=t[:], in_=fm_ap(xd)[:, :, 0:L5CW])
                pre_xt.append(t)

            idm_sb = const.tile([128, 128], MT, tag="idm", name="idmsb")
            nc.sync.dma_start(out=idm_sb[:], in_=idm_d.ap())
            w2_sb = []
            for i, wd in enumerate(w2_d):
                t = const.tile([128, 8, D], MT, tag=f"w2_{i}", name=f"w2sb{i}")
                wap = wd.ap().rearrange("(kt p) e -> p kt e", p=128)
                # two halves so the first matmuls' weights (kt 0-3) land early
                nc.sync.dma_start(out=t[:, 0:4, :], in_=wap[:, 0:4, :])
                nc.sync.dma_start(out=t[:, 4:8, :], in_=wap[:, 4:8, :])
                w2_sb.append(t)
            xt1_sb = []
            for i, xd in enumerate(xt1_d):
                t = const.tile([128, NT, 4], XDT,
                               tag=f"xt1_{i}", name=f"xt1sb{i}")
                nc.sync.dma_start(out=t[:], in_=fm_ap(xd))
                xt1_sb.append(t)
            if split:
                xt0nm_sb = const.tile([1, D], FP32, tag="xt0nm", name="xt0nmsb")
                nc.sync.dma_start(out=xt0nm_sb[:], in_=xt0nm_d.ap())
            else:
                xt0_sb = []
                for i, xd in enumerate(xt0_d):
                    t = const.tile([128, NT, 1], XDT,
                                   tag=f"xt0_{i}", name=f"xt0sb{i}")
                    nc.sync.dma_start(out=t[:], in_=fm_ap(xd))
                    xt0_sb.append(t)

            # each level's h lives in per-chunk tiles so readers depend only
            # on the chunks they actually consume (precise RAW deps)
            h_tiles = {L: {} for L in range(6)}  # L -> {chunk_index: tile}
            h_cw = {}

            def weighted_sums(Hb, w, nsub=1):
                """Ul = 3*H0 + 2*H1 + H2 ; Ur = H1 + 2*H2 + 3*H3.

                Returns [(Ul_part, Ur_part), ...] — one pair per matmul term
                operand: bf16/fp32 -> [(Ul, Ur)]; fp32x -> [(hi), (lo)].

                DVE mode notes: tensor_scalar runs 4x (bf16) / 2x (fp32 SBUF),
                tensor_tensor runs 2x (bf16) / 1x, scalar_tensor_tensor 1x.
                Big chunks use ts+tt; small ones stt (fewer ops = less latency).
                """
                Ul = up.tile([128, NT, w], DT, tag="Ul", name="Ul")
                Ur = up.tile([128, NT, w], DT, tag="Ur", name="Ur")
                tA = tmpp.tile([128, NT, CHUNK], DT, tag="tA", name="tA")
                tB = tmpp.tile([128, NT, CHUNK], DT, tag="tB", name="tB")
                if w >= 256:
                    sw = w // nsub
                    for s in range(nsub):
                        sl = slice(s * sw, (s + 1) * sw)
                        Hs = lambda b: Hb(b)[:, :, sl]  # noqa: B023,E731
                        t1, t2 = tA[:, :, sl], tB[:, :, sl]
                        nc.vector.tensor_scalar_mul(t1, Hs(0), 3.0)
                        nc.vector.tensor_scalar_mul(t2, Hs(1), 2.0)
                        nc.vector.tensor_add(t1, t1, t2)           # 3h0+2h1
                        nc.vector.tensor_add(Ul[:, :, sl], t1, Hs(2))  # +h2
                        t3, t4 = tA[:, :, sl], tB[:, :, sl]
                        nc.vector.tensor_scalar_mul(t3, Hs(3), 3.0)
                        nc.vector.tensor_scalar_mul(t4, Hs(2), 2.0)
                        nc.vector.tensor_add(t3, t3, t4)           # 2h2+3h3
                        nc.vector.tensor_add(Ur[:, :, sl], t3, Hs(1))  # +h1
                else:
                    nc.vector.scalar_tensor_tensor(
                        out=tA[:, :, :w], in0=Hb(0), scalar=3.0, in1=Hb(2),
                        op0=mult, op1=add,
                    )
                    nc.vector.scalar_tensor_tensor(
                        out=Ul[:], in0=Hb(1), scalar=2.0, in1=tA[:, :, :w],
                        op0=mult, op1=add,
                    )
                    nc.vector.scalar_tensor_tensor(
                        out=tB[:, :, :w], in0=Hb(3), scalar=3.0, in1=Hb(1),
                        op0=mult, op1=add,
                    )
                    nc.vector.scalar_tensor_tensor(
                        out=Ur[:], in0=Hb(2), scalar=2.0, in1=tB[:, :, :w],
                        op0=mult, op1=add,
                    )
                if not split:
                    return [(Ul, Ur)]
                # split into MT hi (ScalarE cast) + lo (GpSimd subtract)
                Ulh = usp.tile([128, NT, w], MT, tag="Ulh", name="Ulh")
                Urh = usp.tile([128, NT, w], MT, tag="Urh", name="Urh")
                Ull = usp.tile([128, NT, w], MT, tag="Ull", name="Ull")
                Url = usp.tile([128, NT, w], MT, tag="Url", name="Url")
                nc.scalar.copy(out=Ulh[:], in_=Ul[:])
                nc.gpsimd.tensor_sub(Ull[:], Ul[:], Ulh[:])
                nc.scalar.copy(out=Urh[:], in_=Ur[:])
                nc.gpsimd.tensor_sub(Url[:], Ur[:], Urh[:])
                return [(Ulh, Urh), (Ull, Url)]

            def mm_terms(uparts):
                """(weight_tile, U_part) pairs in accumulation order."""
                if not split:
                    return [(w2_sb[0], uparts[0])]
                (uh, ul) = uparts
                return [(w2_sb[0], uh), (w2_sb[1], uh), (w2_sb[0], ul)]

            def level_matmuls(uparts, x_fns, h_out_fn):
                """agg = x + sum_terms W.T @ [Ul;Ur] (PSUM), h_out = tanh(agg).
                Non-split: x identity-injected on the PE first (no DVE dep).
                Split: x added into PSUM by the DVE after the matmul group."""
                w = uparts[0][0].shape[-1]
                terms = mm_terms(uparts)
                for et in range(NT):
                    ps = psum.tile([128, CHUNK], FP32, tag="agg", name="ps")
                    if not split:
                        for i, xf in enumerate(x_fns):
                            nc.tensor.matmul(
                                ps[:, :w], idm_sb[:], xf(et),
                                start=(i == 0), stop=False,
                            )
                    for ti, (wt, (Ul, Ur)) in enumerate(terms):
                        for kt in range(8):
                            nc.tensor.matmul(
                                ps[:, :w],
                                wt[:, kt, et * 128:(et + 1) * 128],
                                (Ul if kt < 4 else Ur)[:, kt % 4, :],
                                start=(split and ti == 0 and kt == 0),
                                stop=(ti == len(terms) - 1 and kt == 7),
                            )
                    if split:
                        nc.vector.tensor_add(ps[:, :w], ps[:, :w], x_fns[0](et))
                    nc.scalar.activation(out=h_out_fn(et), in_=ps[:, :w], func=Tanh)

            # ---- levels 5..0 (global 7..2) ----
            # emission order IS each engine's execution order, so interleave:
            # L4 chunk 0 right after the even L5 chunks it depends on
            for L in range(6):
                cw = L5CW if L == 5 else 256 if L == 4 else CHUNK
                h_cw[L] = min(cw, NLOC[L])
            nl5 = NLOC[5] // L5CW
            if DT != FP32 and L5CW == 256:
                # interleave (needs slack in the tile pools): L4 chunk k after
                # the even/odd L5 chunks it reads
                schedule = [(5, ci) for ci in range(0, nl5, 2)] + [(4, 0)] \
                    + [(5, ci) for ci in range(1, nl5, 2)] + [(4, 1)]
            else:
                schedule = [(5, ci) for ci in range(nl5)] + [(4, 0), (4, 1)]
            schedule += [(L, 0) for L in range(3, -1, -1)]
            for L, ci in schedule:
                n_out = NLOC[L]
                cw = L5CW if L == 5 else 256 if L == 4 else CHUNK
                if True:
                    w = min(cw, n_out - ci * cw)
                    c0 = ci * cw
                    if L == 5 and ci == 0:
                        lts, xts = pre_lts, pre_xt
                    else:
                        if L == 5:
                            lts = []
                            for b in range(B):
                                lt = leafp.tile(
                                    [128, NT, w], DT, tag="lb", name=f"lb{b}"
                                )
                                nc.sync.dma_start(
                                    out=lt[:], in_=fm_ap(leaf_d[b])[:, :, c0:c0 + w]
                                )
                                lts.append(lt)
                        xts = []
                        for i, xd in enumerate(x_d[L]):
                            t = xsp.tile([128, NT, w], XDT,
                                         tag=f"xs{i}", name=f"xs{i}")
                            nc.sync.dma_start(out=t[:], in_=fm_ap(xd)[:, :, c0:c0 + w])
                            xts.append(t)
                    if L == 5:
                        Hb = lambda b: lts[b][:]  # noqa: B023,E731
                    else:
                        def Hb(b, L=L, c0=c0, w=w):
                            start = b * NLOC[L] + c0
                            j, off = divmod(start, h_cw[L + 1])
                            return h_tiles[L + 1][j][:, :, off:off + w]
                    ht = hp.tile([128, NT, w], DT, tag=f"h{L}_{ci}",
                                 name=f"h{L}_{ci}")
                    h_tiles[L][ci] = ht
                    uparts = weighted_sums(Hb, w)
                    level_matmuls(
                        uparts,
                        [lambda et, t=t: t[:, et, :] for t in xts],
                        lambda et: ht[:, et, :],  # noqa: B023
                    )

            # ---- tail: AllGather the 16 level-2 hiddens, redundant levels 1+0 ----
            # Transposeless gather in the STORAGE dtype (AllGather just moves
            # bytes): ship the feature-major [512, 2] pair, axis-0 concat makes
            # per-core blocks, one strided DMA reassembles [128, ct=(4c+t), r].
            cc_in = dram.tile([D, 2], DT, tag="cc_in", name="cc_in")
            cc_out = dram.tile([NCORES * D, 2], DT, tag="cc_out", name="cc_out")
            nc.sync.dma_start(
                out=cc_in[:, :].rearrange("(t p) r -> p t r", p=128),
                in_=h_tiles[0][0][:],
            )
            if with_tail:
                nc.gpsimd.collective_compute(
                    "AllGather",
                    mybir.AluOpType.bypass,
                    replica_groups=[list(range(NCORES))],
                    ins=[cc_in.opt()],
                    outs=[cc_out.opt()],
                )
            else:  # collective-free variant for single-core cost simulation
                nc.sync.dma_start(out=cc_out[0:D, :], in_=cc_in[:, :])
            h2g_fm = const.tile([128, NT, 16], DT, tag="h2gfm", name="h2gfm")
            cc_out_r = cc_out[:, :].rearrange("(c t p) r -> p t c r", p=128, t=NT)
            for t in range(NT):  # engine/DMA APs are limited to 3 dims
                nc.sync.dma_start(
                    out=h2g_fm[:, t, :].rearrange("p (c r) -> p c r", r=2),
                    in_=cc_out_r[:, t],
                )

            def H1b(b):
                return h2g_fm[:, :, 4 * b:4 * b + 4]

            # level 1 (4 nodes)
            h1_fm = const.tile([128, NT, 4], DT, tag="h1fm", name="h1fm")
            u1 = weighted_sums(H1b, 4)
            level_matmuls(
                u1,
                [lambda et, t=t: t[:, et, :] for t in xt1_sb],
                lambda et: h1_fm[:, et, :],
            )

            # root (node-major directly: out [1, 512])
            u0 = weighted_sums(lambda b: h1_fm[:, :, b:b + 1], 1)
            ps_r = psum.tile([1, D], FP32, tag="agg", name="psr")
            if not split:
                for i, xs in enumerate(xt0_sb):
                    for t in range(NT):
                        nc.tensor.matmul(
                            ps_r[0:1, t * 128:(t + 1) * 128], xs[:, t, :], idm_sb[:],
                            start=(i == 0 and t == 0), stop=False,
                        )
            terms = mm_terms(u0)
            for ti, (wt, (Ul, Ur)) in enumerate(terms):
                for kt in range(8):
                    nc.tensor.matmul(
                        ps_r[0:1, :],
                        (Ul if kt < 4 else Ur)[:, kt % 4, :],
                        wt[:, kt, :],
                        start=(split and ti == 0 and kt == 0),
                        stop=(ti == len(terms) - 1 and kt == 7),
                    )
            if split:
                nc.vector.tensor_add(ps_r[:], ps_r[:], xt0nm_sb[:])
            root_sb = const.tile([1, D], FP32, tag="root", name="rootsb")
            nc.scalar.activation(out=root_sb[:], in_=ps_r[:], func=Tanh)
            nc.sync.dma_start(out=out_d.ap(), in_=root_sb[:])

    nc.compile()
    return nc


_NC_CACHE = {}


def _get_nc():
    key = PRECISION
    if key not in _NC_CACHE:
        _NC_CACHE[key] = _build_nc()
    return _NC_CACHE[key]


# ---------------------------------------------------------------- entry point
def kernel(vectors, wl, wr, branching, depth):
    out, _ = _run(vectors, wl, wr, branching, depth, trace=False)
    return out


def _run(vectors, wl, wr, branching, depth, trace=False):
    assert int(branching) == B and int(depth) == DEPTH
    import time as _time

    in_maps = _make_in_maps(vectors, wl, wr)
    nc = _get_nc()
    last = None
    for attempt in range(6):
        try:
            res = run_bass_kernel_spmd(
                nc, in_maps, core_ids=list(range(NCORES)), trace=trace
            )
            break
        except Exception as e:
            # transient device errors (e.g. NRT_EXEC_UNIT_UNRECOVERABLE left
            # by an interrupted earlier session) clear after a reset cycle,
            # which can take tens of seconds
            last = e
            _time.sleep(5.0 * (attempt + 1))
    else:
        raise last
    return np.asarray(res.results[0]["out"], dtype=np.float32), res


def _make_in_maps(vectors, wl, wr):
    split, _, _, st_dt, mm_dt, x_dt = _mode_cfg()
    vectors = np.asarray(vectors, dtype=np.float32)

    off = [(B**l - 1) // (B - 1) for l in range(DEPTH + 1)]

    def fm(rows, dt):
        return np.ascontiguousarray(rows.T, dtype=dt)

    base = {}
    # W2 = [wl.T ; wr.T] / 3 : agg = W2.T @ [u_l' ; u_r']
    w2 = np.concatenate([np.asarray(wl).T, np.asarray(wr).T], axis=0) / 3.0
    w2 = np.ascontiguousarray(w2, dtype=np.float32)
    if split:
        base["w2hi"], base["w2lo"] = _hi_lo(w2, mm_dt)
    else:
        base["w2"] = w2.astype(mm_dt)
    base["identm"] = np.eye(128, dtype=mm_dt)

    def xput(m, name, rows):
        m[name] = fm(rows, x_dt)

    xput(base, "xt1", vectors[off[1]:off[1] + 4])
    if split:
        base["xt0nm"] = np.ascontiguousarray(
            vectors[off[0]:off[0] + 1], dtype=np.float32
        )
    else:
        xput(base, "xt0", vectors[off[0]:off[0] + 1])

    # core c owns the two global level-2 subtrees with b-major storage
    # positions {2c, 2c+1}: pos = 4*b + p for global level-2 node j = 4p + b.
    g2 = np.arange(16, dtype=np.int64)
    pos = 4 * (g2 % 4) + (g2 // 4)
    inv2 = np.empty(16, dtype=np.int64)
    inv2[pos] = g2  # global level-2 node at each storage position

    in_maps = []
    for c in range(NCORES):
        roots = inv2[2 * c:2 * c + 2]  # [r=0, r=1] global level-2 nodes
        m = dict(base)
        for L in range(7):
            gl = L + 2
            n1 = 4**L  # nodes per subtree at this level
            stor = _storage_nodes(L)  # local node at each storage col
            r, q = stor >> (2 * L), stor & (n1 - 1)
            grows = off[gl] + roots[r] * n1 + q  # global row ids, storage order
            if L == 6:
                arr = fm(vectors[grows], st_dt)
                nq = NLOC[6] // 4
                for b in range(B):
                    m[f"leaf{b}"] = np.ascontiguousarray(arr[:, b * nq:(b + 1) * nq])
            else:
                xput(m, f"x{L}", vectors[grows])
        in_maps.append(m)
    return in_maps


if __name__ == "__main__":
    sys.path.insert(0, "/root/problem")
    d = np.load("/root/problem/ref_cache.npz")
    out = kernel(d["vectors"], d["wl"], d["wr"], 4, 8)
    exp = d["expected"]
    rel = np.linalg.norm(out - exp) / np.linalg.norm(exp)
    print("out[0,:5]:", out[0, :5])
    print("rel:", rel, "absmax:", np.abs(out - exp).max())


# revision 57
# speedup vs baseline: 1.0008x; 1.0008x over previous
"""CBTree (levelized complete 4-ary tree, depth 8, D=512) forward on 8 TRN2 NeuronCores.

Strategy
--------
Math: per level,  h = tanh(x + sum_b lc[b]*(h_b @ wl.T) + rc[b]*(h_b @ wr.T)).
By linearity the per-child matmuls collapse to two matmuls of weighted child
sums:  agg = u_l' @ (wl.T/3) + u_r' @ (wr.T/3)  with integer-coefficient sums
u_l' = 3 h0 + 2 h1 + h2 and u_r' = h1 + 2 h2 + 3 h3  (4x FLOP reduction).

Sharding: the 16 level-2 subtrees are sharded 2-per-core (b-major order so the
post-gather tail stays contiguous); each core runs levels 7..2 of its forest
locally, one AllGather collects the 16 level-2 hiddens, and every core
redundantly computes the tiny levels 1+0 tail.

Layout: everything on-chip is feature-major ([128 partitions, 4 d-tiles, n])
and node storage at every level is digit-reversed (base-4), which makes each
child block b a contiguous column range. The host pre-permutes/transposes/
casts inputs accordingly. x is injected into PSUM via an identity-weight
matmul so ScalarE only has to run tanh(PSUM)->SBUF.

Precision modes (PE runs fp16/bf16 at 1 cyc/row, fp32 at 4):
  f16   - everything fp16 (fastest, rel err ~1e-3)
  bf16  - everything bf16 (same speed, rel err ~9e-3)
  f16x  - fp32 storage, matmuls as 3-term split-fp16 products
          (U ~ Uhi+Ulo, W ~ Whi+Wlo; drop Ulo*Wlo): rel err ~3e-6
  fp32  - native fp32 matmuls: rel err ~5e-7
"""
import sys

import numpy as np
import ml_dtypes

sys.path.insert(0, "/opt/trn_rl_repo")

import concourse.bass as bass  # noqa: E402,F401
import concourse.bacc as bacc  # noqa: E402
import concourse.tile as tile  # noqa: E402
from concourse import mybir  # noqa: E402
from concourse.bass_utils import run_bass_kernel_spmd  # noqa: E402

NCORES = 8
D = 512
NT = 4  # d-tiles of 128
B = 4
DEPTH = 8
# local levels L=0..6 <-> global levels 2..8 ; per-core node counts
NLOC = [2 * 4**l for l in range(7)]  # [2, 8, 32, 128, 512, 2048, 8192]
CHUNK = 512
L5CW = 256

# precision of the on-device pipeline: "f16" | "bf16" | "f16x" | "fp32"
PRECISION = "f16"

BF16np = ml_dtypes.bfloat16
FP32 = mybir.dt.float32
BF16 = mybir.dt.bfloat16
F16 = mybir.dt.float16


# ---------------------------------------------------------------- host helpers
def _rev4(idx: np.ndarray, ndig: int) -> np.ndarray:
    r = np.zeros_like(idx)
    q = idx.copy()
    for _ in range(ndig):
        r = (r << 2) | (q & 3)
        q >>= 2
    return r


def _storage_nodes(level: int) -> np.ndarray:
    """Local node index stored at each storage column of local level ``level``.

    Local node j = r*4^L + q (r = which of the core's two subtrees) is stored
    at column 2*rev4(q) + r; returns the inverse map.
    """
    n = NLOC[level]
    j = np.arange(n, dtype=np.int64)
    r, q = j >> (2 * level), j & ((1 << (2 * level)) - 1)
    pos = 2 * _rev4(q, level) + r
    inv = np.empty(n, dtype=np.int64)
    inv[pos] = j
    return inv


def _mode_cfg():
    """-> (split, DT storage, MT matmul-operand, np storage, np mm, np x)."""
    m = PRECISION
    if m == "f16":
        return False, F16, F16, np.float16, np.float16, np.float16
    if m == "bf16":
        return False, BF16, BF16, BF16np, BF16np, BF16np
    if m == "f16x":
        return True, FP32, F16, np.float32, np.float16, np.float32
    return False, FP32, FP32, np.float32, np.float32, np.float32


def _hi_lo(a: np.ndarray, dt):
    hi = a.astype(dt)
    lo = (a - hi.astype(np.float32)).astype(dt)
    return hi, lo


# ---------------------------------------------------------------- device build
def _build_nc(with_tail=True):
    split, DT, MT, _, _, _ = _mode_cfg()
    nc = bacc.Bacc(
        "TRN2", target_bir_lowering=False, debug=False, num_devices=NCORES
    )

    leaf_d = [
        nc.dram_tensor(f"leaf{b}", [D, NLOC[6] // 4], DT, kind="ExternalInput")
        for b in range(B)
    ]

    # in split mode x stays a single fp32 tensor: the DVE adds it into PSUM
    # (no PE identity-injection, no hi/lo marshalling)
    XDT = FP32 if split else DT

    def xin(name, n):
        return [nc.dram_tensor(name, [D, n], XDT, kind="ExternalInput")]

    x_d = {L: xin(f"x{L}", NLOC[L]) for L in range(6)}
    xt1_d = xin("xt1", 4)
    xt0nm_d = (nc.dram_tensor("xt0nm", [1, D], FP32, kind="ExternalInput")
               if split else None)
    xt0_d = None if split else xin("xt0", 1)
    if split:
        w2_d = [
            nc.dram_tensor("w2hi", [2 * D, D], MT, kind="ExternalInput"),
            nc.dram_tensor("w2lo", [2 * D, D], MT, kind="ExternalInput"),
        ]
    else:
        w2_d = [nc.dram_tensor("w2", [2 * D, D], MT, kind="ExternalInput")]
    idm_d = nc.dram_tensor("identm", [128, 128], MT, kind="ExternalInput")
    out_d = nc.dram_tensor("out", [1, D], FP32, kind="ExternalOutput")

    def fm_ap(handle):
        # DRAM [512, n] -> [128p, 4t, n] with feature d = t*128 + p
        return handle.ap().rearrange("(t p) n -> p t n", p=128)

    mult, add = mybir.AluOpType.mult, mybir.AluOpType.add
    Tanh = mybir.ActivationFunctionType.Tanh

    with tile.TileContext(nc) as tc:
        with (
            tc.tile_pool(name="const", bufs=1) as const,
            tc.tile_pool(name="hp", bufs=1) as hp,
            tc.tile_pool(name="leafp", bufs=12 if DT != FP32 else 8) as leafp,
            tc.tile_pool(name="up", bufs=8 if DT != FP32 else 2) as up,
            tc.tile_pool(name="usp", bufs=2) as usp,
            tc.tile_pool(name="tmpp", bufs=8 if DT != FP32 else 2) as tmpp,
            tc.tile_pool(name="xsp", bufs=4 if DT != FP32 else 3) as xsp,
            tc.tile_pool(name="psum", bufs=8, space="PSUM") as psum,
            tc.tile_pool(name="dram", bufs=1, space="DRAM") as dram,
        ):
            # prefetch the first leaf chunk + its x slice before anything else
            # so the DVE/PE pipeline starts as early as possible
            pre_lts = []
            for b in range(B):
                lt = leafp.tile([128, NT, L5CW], DT, tag="lb", name=f"plb{b}")
                nc.sync.dma_start(out=lt[:], in_=fm_ap(leaf_d[b])[:, :, 0:L5CW])
                pre_lts.append(lt)
            pre_xt = []
            for i, xd in enumerate(x_d[5]):
                t = xsp.tile([128, NT, L5CW], XDT,
                             tag=f"xs{i}", name=f"pxs{i}")
                nc.sync.dma_start(out=t[:], in_=fm_ap(xd)[:, :, 0:L5CW])
                pre_xt.append(t)

            idm_sb = const.tile([128, 128], MT, tag="idm", name="idmsb")
            nc.sync.dma_start(out=idm_sb[:], in_=idm_d.ap())
            w2_sb = []
            for i, wd in enumerate(w2_d):
                t = const.tile([128, 8, D], MT, tag=f"w2_{i}", name=f"w2sb{i}")
                wap = wd.ap().rearrange("(kt p) e -> p kt e", p=128)
                # two halves so the first matmuls' weights (kt 0-3) land early
                nc.sync.dma_start(out=t[:, 0:4, :], in_=wap[:, 0:4, :])
                nc.sync.dma_start(out=t[:, 4:8, :], in_=wap[:, 4:8, :])
                w2_sb.append(t)
            xt1_sb = []
            for i, xd in enumerate(xt1_d):
                t = const.tile([128, NT, 4], XDT,
                               tag=f"xt1_{i}", name=f"xt1sb{i}")
                nc.sync.dma_start(out=t[:], in_=fm_ap(xd))
                xt1_sb.append(t)
            if split:
                xt0nm_sb = const.tile([1, D], FP32, tag="xt0nm", name="xt0nmsb")
                nc.sync.dma_start(out=xt0nm_sb[:], in_=xt0nm_d.ap())
            else:
                xt0_sb = []
                for i, xd in enumerate(xt0_d):
                    t = const.tile([128, NT, 1], XDT,
                                   tag=f"xt0_{i}", name=f"xt0sb{i}")
                    nc.sync.dma_start(out=t[:], in_=fm_ap(xd))
                    xt0_sb.append(t)

            # each level's h lives in per-chunk tiles so readers depend only
            # on the chunks they actually consume (precise RAW deps)
            h_tiles = {L: {} for L in range(6)}  # L -> {chunk_index: tile}
            h_cw = {}

            def weighted_sums(Hb, w, nsub=1):
                """Ul = 3*H0 + 2*H1 + H2 ; Ur = H1 + 2*H2 + 3*H3.

                Returns [(Ul_part, Ur_part), ...] — one pair per matmul term
                operand: bf16/fp32 -> [(Ul, Ur)]; fp32x -> [(hi), (lo)].

                DVE mode notes: tensor_scalar runs 4x (bf16) / 2x (fp32 SBUF),
                tensor_tensor runs 2x (bf16) / 1x, scalar_tensor_tensor 1x.
                Big chunks use ts+tt; small ones stt (fewer ops = less latency).
                """
                Ul = up.tile([128, NT, w], DT, tag="Ul", name="Ul")
                Ur = up.tile([128, NT, w], DT, tag="Ur", name="Ur")
                tA = tmpp.tile([128, NT, CHUNK], DT, tag="tA", name="tA")
                tB = tmpp.tile([128, NT, CHUNK], DT, tag="tB", name="tB")
                if w >= 256:
                    sw = w // nsub
                    for s in range(nsub):
                        sl = slice(s * sw, (s + 1) * sw)
                        Hs = lambda b: Hb(b)[:, :, sl]  # noqa: B023,E731
                        t1, t2 = tA[:, :, sl], tB[:, :, sl]
                        nc.vector.tensor_scalar_mul(t1, Hs(0), 3.0)
                        nc.vector.tensor_scalar_mul(t2, Hs(1), 2.0)
                        nc.vector.tensor_add(t1, t1, t2)           # 3h0+2h1
                        nc.vector.tensor_add(Ul[:, :, sl], t1, Hs(2))  # +h2
                        t3, t4 = tA[:, :, sl], tB[:, :, sl]
                        nc.vector.tensor_scalar_mul(t3, Hs(3), 3.0)
                        nc.vector.tensor_scalar_mul(t4, Hs(2), 2.0)
                        nc.vector.tensor_add(t3, t3, t4)           # 2h2+3h3
                        nc.vector.tensor_add(Ur[:, :, sl], t3, Hs(1))  # +h1
                else:
                    nc.vector.scalar_tensor_tensor(
                        out=tA[:, :, :w], in0=Hb(0), scalar=3.0, in1=Hb(2),
                        op0=mult, op1=add,
                    )
                    nc.vector.scalar_tensor_tensor(
                        out=Ul[:], in0=Hb(1), scalar=2.0, in1=tA[:, :, :w],
                        op0=mult, op1=add,
                    )
                    nc.vector.scalar_tensor_tensor(
                        out=tB[:, :, :w], in0=Hb(3), scalar=3.0, in1=Hb(1),
                        op0=mult, op1=add,
                    )
                    nc.vector.scalar_tensor_tensor(
                        out=Ur[:], in0=Hb(2), scalar=2.0, in1=tB[:, :, :w],
                        op0=mult, op1=add,
                    )
                if not split:
                    return [(Ul, Ur)]
                # split into MT hi (ScalarE cast) + lo (GpSimd subtract)
                Ulh = usp.tile([128, NT, w], MT, tag="Ulh", name="Ulh")
                Urh = usp.tile([128, NT, w], MT, tag="Urh", name="Urh")
                Ull = usp.tile([128, NT, w], MT, tag="Ull", name="Ull")
                Url = usp.tile([128, NT, w], MT, tag="Url", name="Url")
                nc.scalar.copy(out=Ulh[:], in_=Ul[:])
                nc.gpsimd.tensor_sub(Ull[:], Ul[:], Ulh[:])
                nc.scalar.copy(out=Urh[:], in_=Ur[:])
                nc.gpsimd.tensor_sub(Url[:], Ur[:], Urh[:])
                return [(Ulh, Urh), (Ull, Url)]

            def mm_terms(uparts):
                """(weight_tile, U_part) pairs in accumulation order."""
                if not split:
                    return [(w2_sb[0], uparts[0])]
                (uh, ul) = uparts
                return [(w2_sb[0], uh), (w2_sb[1], uh), (w2_sb[0], ul)]

            def level_matmuls(uparts, x_fns, h_out_fn):
                """agg = x + sum_terms W.T @ [Ul;Ur] (PSUM), h_out = tanh(agg).
                Non-split: x identity-injected on the PE first (no DVE dep).
                Split: x added into PSUM by the DVE after the matmul group."""
                w = uparts[0][0].shape[-1]
                terms = mm_terms(uparts)
                for et in range(NT):
                    ps = psum.tile([128, CHUNK], FP32, tag="agg", name="ps")
                    if not split:
                        for i, xf in enumerate(x_fns):
                            nc.tensor.matmul(
                                ps[:, :w], idm_sb[:], xf(et),
                                start=(i == 0), stop=False,
                            )
                    for ti, (wt, (Ul, Ur)) in enumerate(terms):
                        for kt in range(8):
                            nc.tensor.matmul(
                                ps[:, :w],
                                wt[:, kt, et * 128:(et + 1) * 128],
                                (Ul if kt < 4 else Ur)[:, kt % 4, :],
                                start=(split and ti == 0 and kt == 0),
                                stop=(ti == len(terms) - 1 and kt == 7),
                            )
                    if split:
                        nc.vector.tensor_add(ps[:, :w], ps[:, :w], x_fns[0](et))
                    nc.scalar.activation(out=h_out_fn(et), in_=ps[:, :w], func=Tanh)

            # ---- levels 5..0 (global 7..2) ----
            # emission order IS each engine's execution order, so interleave:
            # L4 chunk 0 right after the even L5 chunks it depends on
            for L in range(6):
                cw = L5CW if L == 5 else 256 if L == 4 else CHUNK
                h_cw[L] = min(cw, NLOC[L])
            nl5 = NLOC[5] // L5CW
            if DT != FP32 and L5CW == 256:
                # interleave (needs slack in the tile pools): L4 chunk k after
                # the even/odd L5 chunks it reads
                schedule = [(5, ci) for ci in range(0, nl5, 2)] + [(4, 0)] \
                    + [(5, ci) for ci in range(1, nl5, 2)] + [(4, 1)]
            else:
                schedule = [(5, ci) for ci in range(nl5)] + [(4, 0), (4, 1)]
            schedule += [(L, 0) for L in range(3, -1, -1)]
            for L, ci in schedule:
                n_out = NLOC[L]
                cw = L5CW if L == 5 else 256 if L == 4 else CHUNK
                if True:
                    w = min(cw, n_out - ci * cw)
                    c0 = ci * cw
                    if L == 5 and ci == 0:
                        lts, xts = pre_lts, pre_xt
                    else:
                        if L == 5:
                            lts = []
                            for b in range(B):
                                lt = leafp.tile(
                                    [128, NT, w], DT, tag="lb", name=f"lb{b}"
                                )
                                nc.sync.dma_start(
                                    out=lt[:], in_=fm_ap(leaf_d[b])[:, :, c0:c0 + w]
                                )
                                lts.append(lt)
                        xts = []
                        for i, xd in enumerate(x_d[L]):
                            t = xsp.tile([128, NT, w], XDT,
                                         tag=f"xs{i}", name=f"xs{i}")
                            nc.sync.dma_start(out=t[:], in_=fm_ap(xd)[:, :, c0:c0 + w])
                            xts.append(t)
                    if L == 5:
                        Hb = lambda b: lts[b][:]  # noqa: B023,E731
                    else:
                        def Hb(b, L=L, c0=c0, w=w):
                            start = b * NLOC[L] + c0
                            j, off = divmod(start, h_cw[L + 1])
                            return h_tiles[L + 1][j][:, :, off:off + w]
                    ht = hp.tile([128, NT, w], DT, tag=f"h{L}_{ci}",
                                 name=f"h{L}_{ci}")
                    h_tiles[L][ci] = ht
                    uparts = weighted_sums(Hb, w)
                    level_matmuls(
                        uparts,
                        [lambda et, t=t: t[:, et, :] for t in xts],
                        lambda et: ht[:, et, :],  # noqa: B023
                    )

            # ---- tail: AllGather the 16 level-2 hiddens, redundant levels 1+0 ----
            # Transposeless gather in the STORAGE dtype (AllGather just moves
            # bytes): ship the feature-major [512, 2] pair, axis-0 concat makes
            # per-core blocks, one strided DMA reassembles [128, ct=(4c+t), r].
            cc_in = dram.tile([D, 2], DT, tag="cc_in", name="cc_in")
            cc_out = dram.tile([NCORES * D, 2], DT, tag="cc_out", name="cc_out")
            nc.sync.dma_start(
                out=cc_in[:, :].rearrange("(t p) r -> p t r", p=128),
                in_=h_tiles[0][0][:],
            )
            if with_tail:
                nc.gpsimd.collective_compute(
                    "AllGather",
                    mybir.AluOpType.bypass,
                    replica_groups=[list(range(NCORES))],
                    ins=[cc_in.opt()],
                    outs=[cc_out.opt()],
                )
            else:  # collective-free variant for single-core cost simulation
                nc.sync.dma_start(out=cc_out[0:D, :], in_=cc_in[:, :])
            h2g_fm = const.tile([128, NT, 16], DT, tag="h2gfm", name="h2gfm")
            cc_out_r = cc_out[:, :].rearrange("(c t p) r -> p t c r", p=128, t=NT)
            for t in range(NT):  # engine/DMA APs are limited to 3 dims
                nc.sync.dma_start(
                    out=h2g_fm[:, t, :].rearrange("p (c r) -> p c r", r=2),
                    in_=cc_out_r[:, t],
                )

            def H1b(b):
                return h2g_fm[:, :, 4 * b:4 * b + 4]

            # level 1 (4 nodes)
            h1_fm = const.tile([128, NT, 4], DT, tag="h1fm", name="h1fm")
            u1 = weighted_sums(H1b, 4)
            level_matmuls(
                u1,
                [lambda et, t=t: t[:, et, :] for t in xt1_sb],
                lambda et: h1_fm[:, et, :],
            )

            # root (node-major directly: out [1, 512])
            u0 = weighted_sums(lambda b: h1_fm[:, :, b:b + 1], 1)
            ps_r = psum.tile([1, D], FP32, tag="agg", name="psr")
            if not split:
                for i, xs in enumerate(xt0_sb):
                    for t in range(NT):
                        nc.tensor.matmul(
                            ps_r[0:1, t * 128:(t + 1) * 128], xs[:, t, :], idm_sb[:],
                            start=(i == 0 and t == 0), stop=False,
                        )
            terms = mm_terms(u0)
            for ti, (wt, (Ul, Ur)) in enumerate(terms):
                for kt in range(8):
                    nc.tensor.matmul(
                        ps_r[0:1, :],
                        (Ul if kt < 4 else Ur)[:, kt % 4, :],
                        wt[:, kt, :],
                        start=(split and ti == 0 and kt == 0),
                        stop=(ti == len(terms) - 1 and kt == 7),
                    )
            if split:
                nc.vector.tensor_add(ps_r[:], ps_r[:], xt0nm_sb[:])
            root_sb = const.tile([1, D], FP32, tag="root", name="rootsb")
            nc.scalar.activation(out=root_sb[:], in_=ps_r[:], func=Tanh)
            nc.sync.dma_start(out=out_d.ap(), in_=root_sb[:])

    nc.compile()
    return nc


_NC_CACHE = {}


def _get_nc():
    key = PRECISION
    if key not in _NC_CACHE:
        _NC_CACHE[key] = _build_nc()
    return _NC_CACHE[key]


# ---------------------------------------------------------------- entry point
def kernel(vectors, wl, wr, branching, depth):
    out, _ = _run(vectors, wl, wr, branching, depth, trace=False)
    return out


def _run(vectors, wl, wr, branching, depth, trace=False):
    assert int(branching) == B and int(depth) == DEPTH
    import time as _time

    in_maps = _make_in_maps(vectors, wl, wr)
    nc = _get_nc()
    last = None
    for attempt in range(6):
        try:
            res = run_bass_kernel_spmd(
                nc, in_maps, core_ids=list(range(NCORES)), trace=trace
            )
            break
        except Exception as e:
            # transient device errors (e.g. NRT_EXEC_UNIT_UNRECOVERABLE left
            # by an interrupted earlier session) clear after a reset cycle,
            # which can take tens of seconds
            last = e
            _time.sleep(5.0 * (attempt + 1))
    else:
        raise last
    return np.asarray(res.results[0]["out"], dtype=np.float32), res


def _make_in_maps(vectors, wl, wr):
    split, _, _, st_dt, mm_dt, x_dt = _mode_cfg()
    vectors = np.asarray(vectors, dtype=np.float32)

    off = [(B**l - 1) // (B - 1) for l in range(DEPTH + 1)]

    def fm(rows, dt):
        return np.ascontiguousarray(rows.T, dtype=dt)

    base = {}
    # W2 = [wl.T ; wr.T] / 3 : agg = W2.T @ [u_l' ; u_r']
    w2 = np.concatenate([np.asarray(wl).T, np.asarray(wr).T], axis=0) / 3.0
    w2 = np.ascontiguousarray(w2, dtype=np.float32)
    if split:
        base["w2hi"], base["w2lo"] = _hi_lo(w2, mm_dt)
    else:
        base["w2"] = w2.astype(mm_dt)
    base["identm"] = np.eye(128, dtype=mm_dt)

    def xput(m, name, rows):
        m[name] = fm(rows, x_dt)

    xput(base, "xt1", vectors[off[1]:off[1] + 4])
    if split:
        base["xt0nm"] = np.ascontiguousarray(
            vectors[off[0]:off[0] + 1], dtype=np.float32
        )
    else:
        xput(base, "xt0", vectors[off[0]:off[0] + 1])

    # core c owns the two global level-2 subtrees with b-major storage
    # positions {2c, 2c+1}: pos = 4*b + p for global level-2 node j = 4p + b.
    g2 = np.arange(16, dtype=np.int64)
    pos = 4 * (g2 % 4) + (g2 // 4)
    inv2 = np.empty(16, dtype=np.int64)
    inv2[pos] = g2  # global level-2 node at each storage position

    in_maps = []
    for c in range(NCORES):
        roots = inv2[2 * c:2 * c + 2]  # [r=0, r=1] global level-2 nodes
        m = dict(base)
        for L in range(7):
            gl = L + 2
            n1 = 4**L  # nodes per subtree at this level
            stor = _storage_nodes(L)  # local node at each storage col
            r, q = stor >> (2 * L), stor & (n1 - 1)
            grows = off[gl] + roots[r] * n1 + q  # global row ids, storage order
            if L == 6:
                arr = fm(vectors[grows], st_dt)
                nq = NLOC[6] // 4
                for b in range(B):
                    m[f"leaf{b}"] = np.ascontiguousarray(arr[:, b * nq:(b + 1) * nq])
            else:
                xput(m, f"x{L}", vectors[grows])
        in_maps.append(m)
    return in_maps


if __name__ == "__main__":
    sys.path.insert(0, "/root/problem")
    d = np.load("/root/problem/ref_cache.npz")
    out = kernel(d["vectors"], d["wl"], d["wr"], 4, 8)
    exp = d["expected"]
    rel = np.linalg.norm(out - exp) / np.linalg.norm(exp)
    print("out[0,:5]:", out[0, :5])
    print("rel:", rel, "absmax:", np.abs(out - exp).max())
